# revision 25
# baseline (speedup 1.0000x reference)
"""Trainium2 Bass kernel for windowed attention with relative-position bias.

Problem (hardcoded shapes):
  x        [16, 1024, 256] f32
  w_qkv    [256, 768]      f32
  w_proj   [256, 256]      f32
  b_proj   [256]           f32
  bias_table [3969, 8]     f32
  out      [16, 1024, 256] f32

Sharding: data-parallel over batch B=16 across 8 cores (2 batches/core).

v2 design (fp8 DoubleRow attention):
  - q/k evacuated to fp8e4m3 (sqrt(SCALE) folded into both), S^T computed
    with DoubleRow matmuls: lhsT = [32 dims, 2 slabs, keys] where slab 1 is
    zeros (DR charges 0.5 cycles/out-col regardless), rhs = q8 [32, 2, 512].
  - raw bias B (not exp'd) added into S psum by a DR "unpack" matmul:
    lhsT = interleaved identity [64, 2, 128], rhs = host-packed fp8 bias
    tiles [64, 2, 512]; out rows 64t+k <- rhs[k, t, :].  One [128, 2, 1024]
    fp8 DMA per (head-pair, m-chunk) covers both heads (rows 0-63 h0,
    64-127 h1).  This removes the baseline's 16 MB gather-DMA stream and
    the per-tile DVE bias multiply.
  - softmax shift: exp(S + B - 1.25) to keep fp8 P in range (shift cancels
    in normalization).  exp runs on ACT (fp8 out) for most tiles; a subset
    is offloaded to DVE as a Schraudolph bit-trick exp (uint8 out =
    S*(8/ln2) + (56 - C - 1.25*8/ln2), bitcast to e4m3) to balance engines.
  - AV with DoubleRow: lhsT = V_aug fp8 [128 keys, 2 pair-slabs, 64], rhs =
    pt pairs [128, 2, 512]; out [64, 512] at psum partitions 0-63 (DR
    forbids dst partition offsets), accumulated over 4 m-pairs per head.
    V_aug col 32 = ones -> av row 32 = softmax denominator.
  - per-head av [64, 1024] psum (tag-shared bufs=1) + S ring bufs=3 fills
    exactly 8 PSUM banks.
  - normalize / output projection as baseline (PE broadcast of denominators
    via ones-block ebc, DVE reciprocal+mult, bf16 matmul + bias add).
"""

import numpy as np
import ml_dtypes
import bass_rust

import concourse.bass as bass
import concourse.mybir as mybir
import concourse.tile as tile
from concourse import bacc
from concourse.bass_utils import run_bass_kernel_spmd

BF16 = mybir.dt.bfloat16
F16 = mybir.dt.float16
F32R = mybir.dt.float32r
F32 = mybir.dt.float32
FP8 = mybir.dt.float8e4
U8 = mybir.dt.uint8
DR = mybir.MatmulPerfMode.DoubleRow

B, N, C = 16, 1024, 256
H, D = 8, 32
SCALE = D ** -0.5
H_GRID = W_GRID = 32
TS = 63
TABLE_SIZE = TS * TS  # 3969
N_CORES = 8
B_PER_CORE = B // N_CORES  # 2

SHIFT = -2.0             # softmax shift, cancels in normalization
SCHR_A = 8.0 / np.log(2.0)
SCHR_B = 8.0 * 7 - 0.458 + SHIFT * SCHR_A
# m-chunks whose exp runs on DVE (Schraudolph) instead of ACT
DVE_EXP_M = set()

_nf16 = np.float16
_e4 = ml_dtypes.float8_e4m3


def build_nc():
    nc = bacc.Bacc("TRN2", target_bir_lowering=False, debug=False,
                   num_devices=N_CORES)

    xt = nc.dram_tensor("xt", [B_PER_CORE, C, N], F16, kind="ExternalInput").ap()
    wqk = nc.dram_tensor("wqk", [C, 512], F16, kind="ExternalInput").ap()
    wv = nc.dram_tensor("wv", [C, 256], F16, kind="ExternalInput").ap()
    wproj = nc.dram_tensor("wproj", [C, 256], F16, kind="ExternalInput").ap()
    bproj = nc.dram_tensor("bproj", [1, 512], F16, kind="ExternalInput").ap()
    # packed raw-bias tiles: [hp, m, 128, 2, 1024] fp8 (rows 0-63 h0 in
    # unpack layout [k, t, n] = B_h0[64t+k, n]; rows 64-127 h1)
    b8 = nc.dram_tensor("b8", [4, 8, 128, 2 * 1024], FP8, kind="ExternalInput").ap()
    # interleaved identity for the unpack matmul: idn[p, t, i] = (i == 64t + p%64)
    idn = nc.dram_tensor("idn", [128, 2 * 128], FP8, kind="ExternalInput").ap()
    # ebc rows 0,32,64,96 all-ones (denominator broadcast lhsT)
    ebc = nc.dram_tensor("ebc", [128, 32], F16, kind="ExternalInput").ap()
    y = nc.dram_tensor("y", [B_PER_CORE, N, C], F16, kind="ExternalOutput").ap()
    import os
    DEBUG_DUMP = bool(int(os.environ.get("K_DEBUG", "0")))
    if DEBUG_DUMP:
        dqk = nc.dram_tensor("dqk", [128, 2 * 4 * N], F16, kind="ExternalOutput").ap()
        dv8 = nc.dram_tensor("dv8", [2, 128, 2 * 4 * 2 * H * 64], FP8, kind="ExternalOutput").ap()
        dpt = nc.dram_tensor("dpt", [2, 128, 4 * 2 * 1024], FP8, kind="ExternalOutput").ap()
        dot = nc.dram_tensor("dot", [128, 2 * 8 * 1024], F32, kind="ExternalOutput").ap()
        ddp = nc.dram_tensor("ddp", [128, 2 * 2 * N], F32, kind="ExternalOutput").ap()

    from contextlib import ExitStack
    with tile.TileContext(nc) as tc, ExitStack() as ctx:
        consts = ctx.enter_context(tc.tile_pool(name="consts", bufs=1))
        persist = ctx.enter_context(tc.tile_pool(name="persist", bufs=1))
        xt_pool = ctx.enter_context(tc.tile_pool(name="xt", bufs=1))
        bias_pool = ctx.enter_context(tc.tile_pool(name="bias", bufs=2))
        pt_pool = ctx.enter_context(tc.tile_pool(name="pt", bufs=2))
        rec_pool = ctx.enter_context(tc.tile_pool(name="rec", bufs=2))
        ysb_pool = ctx.enter_context(tc.tile_pool(name="ysb", bufs=4))
        s_psum = ctx.enter_context(tc.tile_pool(name="spsum", bufs=3, space="PSUM"))
        av_psum = ctx.enter_context(tc.tile_pool(name="avpsum", bufs=1, space="PSUM"))

        # ---- first-block inputs win the DMA queue head ----
        xt_tiles = []
        xt_sb0 = xt_pool.tile([128, 2, N], F16, name="xt0")
        nc.sync.dma_start(xt_sb0[:],
                          xt[0].rearrange("(kc p) n -> p kc n", p=128))
        xt_tiles.append(xt_sb0)
        wqk_sb = consts.tile([128, 2, 512], F16)
        nc.sync.dma_start(wqk_sb[:], wqk.rearrange("(kc p) m -> p kc m", p=128))
        idn_sb = consts.tile([128, 2, 128], FP8)
        nc.scalar.dma_start(idn_sb.rearrange("p a b -> p (a b)"), idn)
        wv_sb = consts.tile([128, 2, 256], F16)
        nc.sync.dma_start(wv_sb[:], wv.rearrange("(kc p) m -> p kc m", p=128))
        ebc_sb = consts.tile([128, 32], F16)
        nc.sync.dma_start(ebc_sb[:], ebc)
        # warm the ACT exp table early (input: first xt columns, value unused)
        actwarm = consts.tile([128, 8], F32)
        nc.scalar.activation(actwarm[:], xt_sb0[:, 0, 0:8],
                             mybir.ActivationFunctionType.Exp)
        ones1 = consts.tile([1, 128], F16)
        nc.gpsimd.memset(ones1[:], 1.0)
        shift_sb = consts.tile([128, 1], F32)
        nc.gpsimd.memset(shift_sb[:], SHIFT)
        wproj_sb = consts.tile([128, 2, 256], F16)
        bproj_sb = consts.tile([1, 512], F16)
        bproj2_sb = consts.tile([128, 512], F32)

        def load_deferred_inputs():
            nc.sync.dma_start(wproj_sb[:],
                              wproj.rearrange("(kc p) c -> p kc c", p=128))
            nc.sync.dma_start(bproj_sb[:], bproj)
            bp_ps = av_psum.tile([128, 512], F32, tag="av0", name="bp_ps")
            nc.tensor.matmul(bp_ps[:], lhsT=ones1[:], rhs=bproj_sb[:],
                             start=True, stop=True)
            nc.vector.tensor_copy(bproj2_sb[:], bp_ps[:])

        # batch 1 input right behind batch 0's
        xt_sb1 = xt_pool.tile([128, 2, N], F16, name="xt1")
        nc.sync.dma_start(xt_sb1[:],
                          xt[1].rearrange("(kc p) n -> p kc n", p=128))
        xt_tiles.append(xt_sb1)
        # reversed-key copies of x^T (keys processed in (ym, 31-xm) order)
        xtk_tiles = []
        for b in range(B_PER_CORE):
            xtk_sb = xt_pool.tile([128, 2, N], F16, name=f"xtk{b}")
            src = xt_tiles[b].rearrange("p kc (ym xm) -> p kc ym xm", xm=32)[
                :, :, :, ::-1]
            nc.vector.tensor_copy(
                xtk_sb.rearrange("p kc (ym xm) -> p kc ym xm", xm=32), src)
            xtk_tiles.append(xtk_sb)

        # persistent per-batch tensors
        # qk_sb bf16: [p, b, mt, N]; mt 0/1 = q hg0/hg1, mt 2/3 = k hg0/hg1
        qk_sb = persist.tile([128, B_PER_CORE, 4, N], F16)
        # V_aug fp8 split: v8hi = e4m3(16*V) (ones col = 16), v8lo =
        # e4m3(16*V - v8hi); AV accumulates v8hi*P + v8lo*P = 16*V*P and the
        # factor 16 cancels in normalization (denominator also x16).
        # layout [p(keys in chunk), b, pair, slab, head, 64]
        v8hi_sb = persist.tile([128, B_PER_CORE, 4, 2, H, 64], FP8)
        v8lo_sb = persist.tile([128, B_PER_CORE, 4, 2, H, 64], FP8)
        ot_remap = persist.tile([128, B_PER_CORE, 2, N], F32)
        ot_n = persist.tile([128, B_PER_CORE, 2, N], F16)
        dpack = persist.tile([128, B_PER_CORE, 2, N], F32)
        dpack16 = persist.tile([128, B_PER_CORE, 2, N], F16)
        nc.gpsimd.memset(v8hi_sb[:, :, :, :, :, 32:64], 0.0)
        nc.gpsimd.memset(v8hi_sb[:, :, :, :, :, 32:33], 16.0)
        nc.gpsimd.memset(v8lo_sb[:, :, :, :, :, 32:64], 0.0)
        nc.gpsimd.memset(dpack[:], 0.0)

        # ---- Phase A ----
        def phase_a_qk(b, mts=(0, 2, 1, 3)):
            # psum rows = wqk columns (host-permuted): mt0 = q heads0-3,
            # mt1 = q heads4-7, mt2 = k heads0-3, mt3 = k heads4-7.
            for mt in mts:
                rhs_sb = xt_tiles[b] if mt < 2 else xtk_tiles[b]
                ps = s_psum.tile([128, 1024], F32, tag="sps", name="ps")
                for nchk in range(2):
                    for kc in range(2):
                        nc.tensor.matmul(
                            ps[:, nchk * 512:(nchk + 1) * 512],
                            lhsT=wqk_sb[:, kc, mt * 128:(mt + 1) * 128],
                            rhs=rhs_sb[:, kc, nchk * 512:(nchk + 1) * 512],
                            start=(kc == 0), stop=(kc == 1),
                        )
                nc.vector.tensor_copy(qk_sb[:, b, mt, :], ps[:])

        def phase_a_v(b, groups=(0, 1)):
            xtk_sb = xtk_tiles[b]
            for g in groups:
                vp = s_psum.tile([128, 1024], F32, tag="sps", name="vp")
                for nt in range(4):
                    for kc in range(2):
                        nc.tensor.matmul(
                            vp[:, nt * 256:(nt + 1) * 256],
                            lhsT=xtk_sb[:, kc, (4 * g + nt) * 128:(4 * g + nt + 1) * 128],
                            rhs=wv_sb[:, kc, :],
                            start=(kc == 0), stop=(kc == 1),
                        )
                vsrc = vp.rearrange("p (pr sl h c) -> p pr sl h c", pr=2, sl=2, h=8)
                vhi = v8hi_sb[:, b, 2 * g:2 * g + 2, :, :, 0:32]
                vlo = v8lo_sb[:, b, 2 * g:2 * g + 2, :, :, 0:32]
                nc.vector.tensor_scalar(vhi, vsrc, 16.0, None,
                                        mybir.AluOpType.mult)
                nc.vector.scalar_tensor_tensor(vlo, vsrc, 16.0, vhi,
                                               mybir.AluOpType.mult,
                                               mybir.AluOpType.subtract)

        def load_bias_tile(hp, m, eng=None):
            btile = bias_pool.tile([128, 2, 1024], FP8,
                                   tag=f"bias_{m % 4}", name=f"bias_{m % 4}")
            (eng or nc.sync).dma_start(
                btile.rearrange("p a b -> p (a b)"), b8[hp, m])
            return btile

        def attention_block(hp, b, bias_tiles, after_m=None):
            pt = {}
            for hi in range(2):
                pt[hi] = pt_pool.tile([128, 4, 2, 1024], FP8,
                                      tag=f"pt{hi}", name=f"pt{hi}")
            for m in range(8):
                for hi in range(2):
                    h = 2 * hp + hi
                    bp = 32 * (h % 4)
                    hg = h // 4
                    sp = s_psum.tile([128, 1024], F32, tag="sps",
                                     name=f"sp{hi}")
                    for nchk in range(2):
                        sl = slice(nchk * 512, (nchk + 1) * 512)
                        nc.tensor.matmul(
                            sp[:, sl],
                            lhsT=qk_sb[bp:bp + 32, b, 2 + hg, m * 128:(m + 1) * 128],
                            rhs=qk_sb[bp:bp + 32, b, hg, sl],
                            start=True, stop=False,
                            tile_position=(bp, 0),
                        )
                        nc.tensor.matmul(
                            sp[:, sl],
                            lhsT=idn_sb[64 * hi:64 * hi + 64, :, :],
                            rhs=bias_tiles[m][64 * hi:64 * hi + 64, :, sl],
                            start=False, stop=True, perf_mode=DR,
                            tile_position=(64 * hi, 0),
                            skip_group_check=True,
                        )
                    ptdst = pt[hi][:, m // 2, m % 2, :]
                    if m in DVE_EXP_M:
                        nc.vector.tensor_scalar(
                            ptdst.bitcast(U8), sp[:], SCHR_A, SCHR_B,
                            mybir.AluOpType.mult, mybir.AluOpType.add)
                    else:
                        nc.scalar.activation(
                            ptdst, sp[:], mybir.ActivationFunctionType.Exp,
                            bias=shift_sb[:])
                if after_m and m in after_m:
                    for fn in after_m[m]:
                        fn()
            return pt

        def emit_av(hp, b, pt, hi, tail=False):
            # AV for one head: 2 col chunks x (v8hi + v8lo) x 4 pair-chunks,
            # out [64, 512] at psum partitions 0-63 (DR needs dst partition 0)
            h = 2 * hp + hi
            ot_t = rec_pool.tile([64, 1024], F32, tag="ot", bufs=4,
                                 name=f"ot{hi}")
            for nchk in range(2):
                sl = slice(nchk * 512, (nchk + 1) * 512)
                av = av_psum.tile([64, 512], F32,
                                  name=f"av{hi}{nchk}", tag=f"av{nchk}")
                for vi, vt in enumerate((v8hi_sb, v8lo_sb)):
                    for pr in range(4):
                        nc.tensor.matmul(
                            av[:],
                            lhsT=vt[:, b, pr, :, h, :],
                            rhs=pt[hi][:, pr, :, sl],
                            start=(vi == 0 and pr == 0),
                            stop=(vi == 1 and pr == 3), perf_mode=DR,
                            skip_group_check=True,
                        )
                nc.vector.tensor_copy(ot_t[:, sl], av[:])
            eng = nc.scalar if (tail and hi == 1) else nc.sync
            eng.dma_start(
                ot_remap[(32 * h) % 128:(32 * h) % 128 + 32, b, h // 4, :],
                ot_t[0:32, :])
            eng.dma_start(
                dpack[(32 * h) % 128:(32 * h) % 128 + 1, b, h // 4, :],
                ot_t[32:33, :])

        def normalize(b, kc, halves=(0, 1), rb=None):
            if rb is None:
                nc.vector.tensor_copy(dpack16[:, b, kc, :], dpack[:, b, kc, :])
            for half in halves:
                sl = slice(half * 512, (half + 1) * 512)
                rp = av_psum.tile([128, 512], F32, tag=f"av{half}", name="rp")
                for k in range(4):
                    nc.tensor.matmul(
                        rp[32 * k:32 * k + 32, :],
                        lhsT=ebc_sb[32 * k:32 * k + 32, :],
                        rhs=dpack16[32 * k:32 * k + 32, b, kc, sl],
                        start=True, stop=True,
                        tile_position=(32 * k, 32 * k),
                        skip_group_check=True,
                    )
                rsb = rec_pool.tile([128, 1024], F32, tag="rsb", name="rsb")
                nc.vector.reciprocal(rsb[:, sl], rp[:])
                nc.vector.tensor_mul(out=ot_n[:, b, kc, sl],
                                     in0=ot_remap[:, b, kc, sl],
                                     in1=rsb[:, sl])
            return True

        def phase_c(b, ntps=(0, 1, 2, 3)):
            y_re = y[b].rearrange("(g p) c -> p g c", p=128)
            for ntp in ntps:
                ysb = ysb_pool.tile([128, 2, 256], F16, name="ysb")
                yp = av_psum.tile([128, 512], F32, tag=f"av{ntp % 2}",
                                  name="yp")
                for sub in range(2):
                    nt = 2 * ntp + sub
                    for kc in range(2):
                        nc.tensor.matmul(
                            yp[:, sub * 256:(sub + 1) * 256],
                            lhsT=ot_n[:, b, kc, nt * 128:(nt + 1) * 128],
                            rhs=wproj_sb[:, kc, :],
                            start=(kc == 0), stop=(kc == 1),
                            skip_group_check=True,
                        )
                nc.vector.tensor_add(
                    out=ysb.rearrange("p g c -> p (g c)"), in0=yp[:],
                    in1=bproj2_sb[:])
                nc.sync.dma_start(y_re[:, 2 * ntp:2 * ntp + 2, :], ysb[:])

        import os
        REPEAT = int(os.environ.get("K_REPEAT", "1"))
        for _rep in range(REPEAT):
            phase_a_qk(0)
            pending = None  # (hp, b, pt) whose AV is deferred into next block
            bias_all = {}
            for hp in range(4):
                bias_tiles = bias_all.setdefault(hp, {})
                for m in range(8):
                    if m not in bias_tiles:
                        eng = nc.scalar if (_rep == 0 and hp == 0 and m < 2) else None
                        bias_tiles[m] = load_bias_tile(hp, m, eng)
                if _rep == 0 and hp == 1:
                    load_deferred_inputs()
                for b in range(B_PER_CORE):
                    after_m = {}
                    if pending is not None:
                        php, pb, ppt = pending
                        after_m[1] = [lambda php=php, pb=pb, ppt=ppt:
                                      emit_av(php, pb, ppt, 0)]
                        after_m[3] = [lambda php=php, pb=pb, ppt=ppt:
                                      emit_av(php, pb, ppt, 1)]
                    if hp == 0 and b == 0:
                        after_m.setdefault(1, []).append(
                            lambda: phase_a_qk(1, (0,)))
                        after_m[2] = [lambda: phase_a_qk(1, (2,))]
                        after_m.setdefault(3, []).append(
                            lambda: phase_a_qk(1, (1,)))
                        after_m[4] = [lambda: phase_a_qk(1, (3,))]
                        after_m[5] = [lambda: phase_a_v(0, (0,))]
                        after_m[6] = [lambda: phase_a_v(0, (1,))]
                    if hp == 0 and b == 1:
                        after_m[5] = [lambda: phase_a_v(1, (0,))]
                        after_m[6] = [lambda: phase_a_v(1, (1,))]
                    if hp == 1 and b == 1:
                        after_m[5] = [lambda: normalize(0, 0)]
                    if hp == 2 and b == 0:
                        after_m[5] = [lambda: normalize(1, 0)]
                    if hp < 3 and b == 1:
                        nxt = bias_all.setdefault(hp + 1, {})

                        def prefetch(ms, hp=hp, nxt=nxt):
                            for m in ms:
                                nxt[m] = load_bias_tile(hp + 1, m)
                        after_m.setdefault(2, []).append(
                            lambda pf=prefetch: pf((0, 1, 2, 3)))
                        after_m.setdefault(4, []).append(
                            lambda pf=prefetch: pf((4, 5, 6, 7)))
                    if hp == 3 and b == 1:
                        after_m[4] = [lambda: normalize(0, 1)]
                        after_m[5] = [lambda: phase_c(0, (0, 1))]
                        after_m[6] = [lambda: phase_c(0, (2, 3))]
                    pt = attention_block(hp, b, bias_tiles, after_m=after_m)
                    if DEBUG_DUMP and hp == 0 and b == 0:
                        for hi in range(2):
                            nc.sync.dma_start(
                                dpt[hi],
                                pt[hi].rearrange("p a b c -> p (a b c)"))
                    pending = (hp, b, pt)
            # tail: last block's AV inline, then kc=1 normalize + projection
            php, pb, ppt = pending
            emit_av(php, pb, ppt, 0)
            emit_av(php, pb, ppt, 1, tail=True)
            if DEBUG_DUMP:
                nc.sync.dma_start(
                    ddp, dpack.rearrange("p a b c -> p (a b c)"))
            rb_t = normalize(1, 1, halves=(0,))
            phase_c(1, (0,))
            normalize(1, 1, halves=(1,), rb=rb_t)
            phase_c(1, (1, 2, 3))

    nc.compile()
    return nc


_NC_CACHE = None


def _get_nc():
    global _NC_CACHE
    if _NC_CACHE is None:
        _NC_CACHE = build_nc()
    return _NC_CACHE


def _bias_pack(bias_table):
    """Raw bias tiles in the DR-unpack layout.

    B_h[p=(ym_l, xm'), n=(yn, xn)] for m-chunk m (keys (4m+ym_l, 31-xm')):
      = tbl[(yn - 4m - ym_l + 31) * 63 + (xn + xm'), h]
    packed as b8[hp, m, 64*hi + k, t, n] = B_{2hp+hi}[64t + k, n].
    """
    tblf = bias_table.astype(np.float32).reshape(TS, TS, H)
    ym_l = np.arange(4)[:, None, None, None]
    xm = np.arange(32)[None, :, None, None]
    yn = np.arange(32)[None, None, :, None]
    xn = np.arange(32)[None, None, None, :]
    out = np.empty((4, 8, 128, 2, 1024), dtype=_e4)
    for m in range(8):
        r = yn - 4 * m - ym_l + 31          # [4,1,32,1] broadcast
        c = xn + xm                          # [1,32,1,32]
        rb, cb = np.broadcast_arrays(r, c)
        for h in range(H):
            bt = tblf[rb, cb, h]             # [4, 32, 32, 32]
            bt = bt.reshape(128, 1024)       # p=(ym_l, xm'), n=(yn, xn)
            # unpack layout [k, t, n] = bt[64t + k, n]
            bt_u = bt.reshape(2, 64, 1024).transpose(1, 0, 2)
            out[h // 2, m, 64 * (h % 2):64 * (h % 2) + 64] = bt_u.astype(_e4)
    return out


def _host_prep(x, w_qkv, w_proj, b_proj, bias_table):
    xt = np.ascontiguousarray(np.transpose(x, (0, 2, 1))).astype(_nf16)  # [B, C, N]
    ss = np.sqrt(SCALE)
    w_q = w_qkv[:, :256] * ss
    w_k = w_qkv[:, 256:512] * ss
    # column order: mt0 = q heads0-3, mt1 = q heads4-7, mt2/3 = k heads0-3/4-7
    w_qk = np.concatenate(
        [w_q[:, :128], w_q[:, 128:], w_k[:, :128], w_k[:, 128:]], axis=1
    ).astype(_nf16)
    w_v = w_qkv[:, 512:].astype(_nf16)
    wproj_arr = w_proj.astype(_nf16)
    bproj_arr = np.concatenate([b_proj, b_proj]).reshape(1, 512).astype(_nf16)
    ebc = np.zeros((128, 32), dtype=_nf16)
    for p in (0, 32, 64, 96):
        ebc[p, :] = 1.0
    b8 = _bias_pack(bias_table).reshape(4, 8, 128, 2048)
    idn = np.zeros((128, 2, 128), dtype=_e4)
    p = np.arange(128)
    for t in range(2):
        idn[p, t, 64 * t + p % 64] = 1.0
    idn = idn.reshape(128, 256)
    return xt, w_qk, w_v, wproj_arr, bproj_arr, b8, idn, ebc


def kernel(x, w_qkv, w_proj, b_proj, bias_table):
    x = np.asarray(x, dtype=np.float32)
    w_qkv = np.asarray(w_qkv, dtype=np.float32)
    w_proj = np.asarray(w_proj, dtype=np.float32)
    b_proj = np.asarray(b_proj, dtype=np.float32)
    bias_table = np.asarray(bias_table, dtype=np.float32)

    xt, w_qk, w_v, wproj_arr, bproj_arr, b8, idn, ebc = _host_prep(
        x, w_qkv, w_proj, b_proj, bias_table)

    nc = _get_nc()
    in_maps = []
    for c in range(N_CORES):
        in_maps.append({
            "xt": xt[B_PER_CORE * c:B_PER_CORE * (c + 1)],
            "wqk": w_qk, "wv": w_v, "wproj": wproj_arr, "bproj": bproj_arr,
            "b8": b8, "idn": idn, "ebc": ebc,
        })
    res = run_bass_kernel_spmd(nc, in_maps, core_ids=list(range(N_CORES)))
    out = np.concatenate([res.results[c]["y"] for c in range(N_CORES)], axis=0)
    return out.astype(np.float32)


if __name__ == "__main__":
    rng = np.random.default_rng(0)
    inputs = {
        "x": rng.standard_normal((B, N, C), dtype=np.float32),
        "w_qkv": (rng.standard_normal((C, 3 * 256), dtype=np.float32) * C ** -0.5),
        "w_proj": (rng.standard_normal((256, C), dtype=np.float32) * 256 ** -0.5),
        "b_proj": np.zeros((C,), dtype=np.float32),
        "bias_table": (rng.standard_normal((TABLE_SIZE, H), dtype=np.float32) * 0.02),
    }
    out = kernel(**inputs)
    print("kernel output", out.shape, out.dtype)
